# revision 2
# baseline (speedup 1.0000x reference)
"""Trainium2 Bass kernel for windowed-style attention with relative position bias.

Shapes (hardcoded): x [4, 2048, 512], H=8 heads, HD=64, rel table [4098, 8].

Sharding: 8 cores = 4 batches x 2 query-halves. Each core computes the full
attention + projection for its 1024 query rows of its batch (keys span all
2048 tokens), so outputs are disjoint row slices -- no collectives.

Device dataflow (per core, identical SPMD program):
  - qT/kT = W @ xT (PE, fp32 in, fp16 out; q pre-scaled by HD^-0.5)
  - V computed in natural [token, d] layout with a ones column appended per
    head (gives the softmax denominator for free in the same matmul)
  - scores are computed transposed (S^T: keys on partitions, queries free),
    so softmax reduction over keys happens inside the PE via the ones column
    and no max-subtraction pass is needed (scores ~ N(0,1), exp is safe)
  - E = exp(S^T) (ACT) * exp(bias)^T (DVE, fp16) -- the relative-position
    bias is applied multiplicatively with a host-precomputed exp'ed table
  - O^T accumulated per head in PSUM, normalized by the PE-broadcast
    reciprocal of the denominator row, projection back to [token, C] (PE)
"""

import sys

sys.path.insert(0, "/opt/trn_rl_repo")

import numpy as np

import concourse.mybir as mybir
import concourse.tile as tile
from concourse import bacc
from concourse.bass import ds, ts
from concourse.bass_utils import run_bass_kernel_spmd

B, N, C, H, HD = 4, 2048, 512, 8, 64
NQ = N // 2
NCORES = 8
SCALE = HD ** -0.5
F32 = mybir.dt.float32
F16 = mybir.dt.float16
EXP = mybir.ActivationFunctionType.Exp
COPY = mybir.ActivationFunctionType.Copy


def build_kernel():
    nc = bacc.Bacc("TRN2", target_bir_lowering=False, debug=False, num_devices=NCORES)

    xT = nc.dram_tensor("xT", [C, N], F32, kind="ExternalInput").ap()
    xqT = nc.dram_tensor("xqT", [C, NQ], F32, kind="ExternalInput").ap()
    wqT = nc.dram_tensor("wqT", [C, C], F32, kind="ExternalInput").ap()
    wkT = nc.dram_tensor("wkT", [C, C], F32, kind="ExternalInput").ap()
    wvT = nc.dram_tensor("wvT", [C, C], F32, kind="ExternalInput").ap()
    wp8 = nc.dram_tensor("wp8", [64, 8, C], F16, kind="ExternalInput").ap()
    bpr = nc.dram_tensor("bpr", [1, C], F32, kind="ExternalInput").ap()
    eb = nc.dram_tensor("eb", [H, 2, 128, 16, 512], F16, kind="ExternalInput").ap()
    out = nc.dram_tensor("out", [NQ, C], F32, kind="ExternalOutput").ap()

    with tile.TileContext(nc) as tc:
        with (
            tc.tile_pool(name="const", bufs=1) as Kc,
            tc.tile_pool(name="ebp", bufs=2) as Keb,
            tc.tile_pool(name="ep", bufs=3) as Kep,
            tc.tile_pool(name="rp", bufs=2) as Krp,
            tc.tile_pool(name="outp", bufs=3) as Kout,
            tc.tile_pool(name="ps", bufs=3, space="PSUM") as Kps,
            tc.tile_pool(name="pso", bufs=2, space="PSUM") as Kpso,
            tc.tile_pool(name="psr", bufs=2, space="PSUM") as Kpsr,
        ):
            xT_s = Kc.tile([128, 4, N], F32, name="xT_s")
            xqT_s = Kc.tile([128, 4, NQ], F32, name="xqT_s")
            wq_s = Kc.tile([128, 4, C], F32, name="wq_s")
            wk_s = Kc.tile([128, 4, C], F32, name="wk_s")
            wv_s = Kc.tile([128, 4, C], F32, name="wv_s")
            wp_s = Kc.tile([64, 8, C], F16, name="wp_s")
            bp_s = Kc.tile([1, C], F32, name="bp_s")
            ones_s = Kc.tile([128, 128], F32, name="ones_s")
            qT_s = Kc.tile([128, 4, NQ], F16, name="qT_s")
            kT_s = Kc.tile([128, 4, N], F16, name="kT_s")
            va_s = Kc.tile([128, 16, H, HD + 1], F16, name="va_s")
            ot_s = Kc.tile([64, 8, NQ], F16, name="ot_s")
            bb_s = Kc.tile([128, C], F32, name="bb_s")

            r128 = lambda ap: ap.rearrange("(po pi) t -> pi po t", pi=128)
            nc.sync.dma_start(xT_s, r128(xT))
            nc.sync.dma_start(xqT_s, r128(xqT))
            nc.sync.dma_start(wq_s, r128(wqT))
            nc.sync.dma_start(wk_s, r128(wkT))
            nc.sync.dma_start(wv_s, r128(wvT))
            nc.sync.dma_start(wp_s, wp8)
            nc.sync.dma_start(bp_s, bpr)
            nc.vector.memset(ones_s, 1.0)

            # broadcast b_proj across partitions via a K=1 matmul
            bb_ps = Kps.tile([128, 512], F32, tag="ps")
            nc.tensor.matmul(
                bb_ps, lhsT=ones_s[0:1, 0:128], rhs=bp_s[0:1, :], start=True, stop=True
            )
            nc.vector.tensor_copy(bb_s, bb_ps)

            # ---- phase A: qT, kT, V ----
            for ot in range(4):
                for cb in range(2):
                    ps = Kps.tile([128, 512], F32, tag="ps")
                    for c in range(4):
                        nc.tensor.matmul(
                            ps,
                            lhsT=wq_s[:, c, ts(ot, 128)],
                            rhs=xqT_s[:, c, ts(cb, 512)],
                            start=(c == 0),
                            stop=(c == 3),
                        )
                    nc.scalar.activation(
                        qT_s[:, ot, ts(cb, 512)], ps, COPY, scale=float(SCALE)
                    )
            for ot in range(4):
                for cb in range(4):
                    ps = Kps.tile([128, 512], F32, tag="ps")
                    for c in range(4):
                        nc.tensor.matmul(
                            ps,
                            lhsT=wk_s[:, c, ts(ot, 128)],
                            rhs=xT_s[:, c, ts(cb, 512)],
                            start=(c == 0),
                            stop=(c == 3),
                        )
                    nc.vector.tensor_copy(kT_s[:, ot, ts(cb, 512)], ps)

            nc.vector.memset(va_s, 1.0)  # ones column survives; V cols overwritten
            for tt in range(16):
                ps = Kps.tile([128, 512], F32, tag="ps")
                for c in range(4):
                    nc.tensor.matmul(
                        ps,
                        lhsT=xT_s[:, c, ts(tt, 128)],
                        rhs=wv_s[:, c, :],
                        start=(c == 0),
                        stop=(c == 3),
                    )
                nc.vector.tensor_copy(
                    va_s[:, tt, :, 0:HD], ps.rearrange("p (h d) -> p h d", h=H)
                )

            # ---- phase B: attention ----
            for nb in range(2):
                for h in range(H):
                    eb_t = Keb.tile([128, 16, 512], F16, tag="eb")
                    nc.sync.dma_start(eb_t, eb[h, nb])
                    o_ps = Kpso.tile([128, 512], F32, tag="o")
                    b0 = (h % 2) * 64
                    for mt in range(16):
                        s_ps = Kps.tile([128, 512], F32, tag="ps")
                        nc.tensor.matmul(
                            s_ps,
                            lhsT=kT_s[b0 : b0 + 64, h // 2, ts(mt, 128)],
                            rhs=qT_s[b0 : b0 + 64, h // 2, ts(nb, 512)],
                            start=True,
                            stop=True,
                        )
                        e_t = Kep.tile([128, 512], F16, tag="e")
                        nc.scalar.activation(e_t, s_ps, EXP)
                        nc.vector.tensor_mul(e_t, e_t, eb_t[:, mt, :])
                        nc.tensor.matmul(
                            o_ps[0 : HD + 1, :],
                            lhsT=va_s[:, mt, h, :],
                            rhs=e_t,
                            start=(mt == 0),
                            stop=(mt == 15),
                        )
                    # normalize: rows 0:64 = O^T, row 64 = denominator
                    r_t = Krp.tile([128, 512], F32, tag="r")
                    nc.vector.reciprocal(r_t[64:65, :], o_ps[64:65, :])
                    rb_ps = Kpsr.tile([128, 512], F32, tag="rb")
                    nc.tensor.matmul(
                        rb_ps[0:64, :],
                        lhsT=ones_s[64:65, 0:64],
                        rhs=r_t[64:65, :],
                        start=True,
                        stop=True,
                    )
                    rb_s = Krp.tile([128, 512], F32, tag="rb_s")
                    nc.scalar.activation(rb_s[0:64, :], rb_ps[0:64, :], COPY)
                    nc.vector.tensor_mul(
                        ot_s[:, h, ts(nb, 512)], o_ps[0:64, :], rb_s[0:64, :]
                    )
                # ---- projection for this query block ----
                for ns in range(4):
                    p_ps = Kps.tile([128, 512], F32, tag="ps")
                    for c8 in range(8):
                        nc.tensor.matmul(
                            p_ps,
                            lhsT=ot_s[:, c8, ds(nb * 512 + ns * 128, 128)],
                            rhs=wp_s[:, c8, :],
                            start=(c8 == 0),
                            stop=(c8 == 7),
                        )
                    o_t = Kout.tile([128, 512], F32, tag="out")
                    nc.vector.tensor_add(o_t, p_ps, bb_s)
                    nc.sync.dma_start(out[ds(nb * 512 + ns * 128, 128), :], o_t)

    nc.compile()
    return nc


_NC = None


def _get_nc():
    global _NC
    if _NC is None:
        _NC = build_kernel()
    return _NC


def _prepare_in_maps(x, w_qkv, rel_bias_table, w_proj, b_proj, mask, rel_idx):
    xf = np.ascontiguousarray(np.asarray(x, dtype=np.float32))
    wf = np.asarray(w_qkv, dtype=np.float32)
    wq = np.ascontiguousarray(wf[0:C].T)
    wk = np.ascontiguousarray(wf[C : 2 * C].T)
    wv = np.ascontiguousarray(wf[2 * C : 3 * C].T)
    wpT = np.asarray(w_proj, dtype=np.float32).T  # [ci, co]
    wp8_a = np.ascontiguousarray(
        wpT.reshape(8, 64, C).transpose(1, 0, 2).astype(np.float16)
    )
    bp = np.ascontiguousarray(np.asarray(b_proj, dtype=np.float32).reshape(1, C))

    # exp'ed relative-position bias, transposed to [head, key, query]
    t_exp = np.exp(np.asarray(rel_bias_table, dtype=np.float32)).astype(np.float16)
    idx = np.asarray(rel_idx)
    lut = t_exp[idx]  # [n, m, H] fp16
    ebt = np.ascontiguousarray(lut.transpose(2, 1, 0))  # [H, m, n]

    mask_a = np.asarray(mask)
    all_true = bool(mask_a.all())

    def eb_half(ebt_b, half):
        sl = ebt_b[:, :, half * NQ : (half + 1) * NQ]  # [H, 2048, 1024]
        arr = sl.reshape(H, 16, 128, 2, 512).transpose(0, 3, 2, 1, 4)
        return np.ascontiguousarray(arr)  # [H, 2, 128, 16, 512]

    eb_shared = None
    if all_true:
        eb_shared = [eb_half(ebt, 0), eb_half(ebt, 1)]

    in_maps = []
    for core in range(NCORES):
        b, half = divmod(core, 2)
        if all_true:
            eb_c = eb_shared[half]
        else:
            ebt_b = ebt * mask_a[b].astype(np.float16)[None, :, None]
            eb_c = eb_half(ebt_b, half)
        in_maps.append(
            {
                "xT": np.ascontiguousarray(xf[b].T),
                "xqT": np.ascontiguousarray(xf[b, half * NQ : (half + 1) * NQ].T),
                "wqT": wq,
                "wkT": wk,
                "wvT": wv,
                "wp8": wp8_a,
                "bpr": bp,
                "eb": eb_c,
            }
        )
    return in_maps


def _run(inputs, trace=False):
    nc = _get_nc()
    in_maps = _prepare_in_maps(**inputs)
    res = run_bass_kernel_spmd(nc, in_maps, core_ids=list(range(NCORES)), trace=trace)
    outp = np.empty((B, N, C), dtype=np.float32)
    for core in range(NCORES):
        b, half = divmod(core, 2)
        outp[b, half * NQ : (half + 1) * NQ] = res.results[core]["out"]
    return outp, res


def kernel(**inputs) -> np.ndarray:
    outp, _ = _run(inputs, trace=False)
    return outp
